# revision 9
# baseline (speedup 1.0000x reference)
"""Binarized BasicBlock (BNN) forward on 8 Trainium2 NeuronCores.

Reference computation (per reference.py):
    xb  = sign(x);  wb = sign(w)
    y1  = conv3x3(xb, wb1, pad=1)
    a1  = hardtanh(bn1(y1))          # only sign(a1) feeds conv2
    y2  = conv3x3(sign(a1), wb2, pad=1)
    out = hardtanh(bn2(y2) + x)

Strategy:
  - Data parallel: batch N=64 -> 8 images per core; weights/BN replicated.
  - Conv as 9 shifted matmuls over a zero-padded 58x58 image held in SBUF,
    contraction over input channels: 256 channels = 2 planes of 128
    partitions contracted in ONE matmul via fp8 DoubleRow perf mode.
  - Binarized operands stored as fp8e4 (+-1, 0 exact); PSUM accumulates
    fp32; sums of +-1 with <=2304 terms are exact integers in fp32.
  - BN folded into the activation op: sign(bn1(y)) = Sign(y*s1 + t1) with
    s1 = g1/sqrt(v1+eps), t1 = b1 - m1*s1 (host-folded, passed as inputs).
  - Final stage: Identity(y2*s2+t2) on ScalarE, then add-residual and
    clip (tensor_scalar min+max) on VectorE.
"""

import sys

try:
    import concourse  # noqa: F401
except ImportError:  # pragma: no cover
    sys.path.insert(0, "/opt/trn_rl_repo")

import numpy as np
import ml_dtypes

import concourse.bacc as bacc
import concourse.tile as tile
import concourse.mybir as mybir
from concourse.bass_utils import run_bass_kernel_spmd

dt = mybir.dt
AF = mybir.ActivationFunctionType
ALU = mybir.AluOpType
PM = mybir.MatmulPerfMode

N_CORES = 8
NPER = 8          # images per core
C = 256
H = W = 56
HW = H * W        # 3136
WP = 58           # padded row width (1 + 56 + 1)
PAD = WP * WP     # 3364 padded image length
PLANE = 3376      # allocated plane length (16B aligned; max rhs slice hits 3366)
RPC = 8           # output rows per matmul chunk
CH = RPC * WP     # 464 = padded span per chunk (per k-plane)
CHU = RPC * W     # 448 = useful matmul free dim (garbage cols skipped)
NCH = H // RPC    # 7 chunks per image
BN_EPS = 1e-5

_CACHE = {}


def _zero_pads(nc, t, q):
    """Zero the padding cells of plane q of a [128, 2, PLANE] tile."""
    v = t[:, q, :]
    nc.gpsimd.memset(v[:, 0:WP], 0.0)                 # top pad row
    nc.gpsimd.memset(v[:, 57 * WP:PLANE], 0.0)        # bottom pad row + tail
    # left/right pad columns for rows 1..56: flat positions 57+58j, 58+58j
    cols = v[:, 57:57 + 57 * WP].rearrange("p (r c) -> p r c", c=WP)
    nc.gpsimd.memset(cols[:, :, 0:2], 0.0)


def _plane_view(t, q):
    return t[:, q, 0:PAD].rearrange("p (r c) -> p r c", c=WP)


def _build():
    nc = bacc.Bacc("TRN2", target_bir_lowering=False, debug=False)

    x_d = nc.dram_tensor("x", [NPER, C, H, W], dt.float32, kind="ExternalInput").ap()
    w1_d = nc.dram_tensor("w1b", [2, 128, 9, C], dt.float8e4, kind="ExternalInput").ap()
    w2_d = nc.dram_tensor("w2b", [2, 128, 9, C], dt.float8e4, kind="ExternalInput").ap()
    s1_d = nc.dram_tensor("s1", [2, 128], dt.float32, kind="ExternalInput").ap()
    t1_d = nc.dram_tensor("t1", [2, 128], dt.float32, kind="ExternalInput").ap()
    s2_d = nc.dram_tensor("s2", [2, 128], dt.float32, kind="ExternalInput").ap()
    t2_d = nc.dram_tensor("t2", [2, 128], dt.float32, kind="ExternalInput").ap()
    out_d = nc.dram_tensor("out", [NPER, C, H, W], dt.float32, kind="ExternalOutput").ap()

    with tile.TileContext(nc) as tc:
        with (
            tc.tile_pool(name="wp", bufs=1) as wp,
            tc.tile_pool(name="xin", bufs=4) as xinp,
            tc.tile_pool(name="xb", bufs=2) as xbp,
            tc.tile_pool(name="ab", bufs=2) as abp,
            tc.tile_pool(name="ost", bufs=3) as ostp,
            tc.tile_pool(name="tmp", bufs=4) as tmpp,
            tc.tile_pool(name="ps", bufs=6, space="PSUM") as psp,
            nc.sbuf_tensor([128, 2 * CHU], dt.float8e4) as warm_in,
            nc.psum_tensor([128, CHU], dt.float32) as warm_ps,
        ):
            # ---- PE warm-up: junk matmuls on scratch data so HAM reaches
            # 8/8 before the first real matmul and the PE never starts cold.
            nc.gpsimd.memset(warm_in[:], 0.0)
            wv = warm_in[:].rearrange("p (k c) -> p k c", k=2)
            for _ in range(26):
                nc.tensor.matmul(
                    warm_ps[:], wv[:, :, 0:128], wv[:],
                    start=True, stop=True, perf_mode=PM.DoubleRow,
                )

            # Weight/BN loads go on compute-engine SWDGE queues so the sync
            # queue starts with image 0's input (the critical path).
            w_sb = []
            for wd, tag, eng in ((w1_d, "w1", nc.scalar), (w2_d, "w2", nc.gpsimd)):
                t = wp.tile([128, 2, 9, C], dt.float8e4, tag=tag)
                eng.dma_start(t[:], wd.rearrange("q p k c -> p q k c"))
                w_sb.append(t)
            bn_sb = []
            for bd, tag in ((s1_d, "s1"), (t1_d, "t1"), (s2_d, "s2"), (t2_d, "t2")):
                t = wp.tile([128, 2], dt.float32, tag=tag)
                nc.gpsimd.dma_start(t[:], bd.rearrange("q p -> p q"))
                bn_sb.append(t)
            s1_sb, t1_sb, s2_sb, t2_sb = bn_sb

            for n in range(NPER):
                # ---- load + binarize input ----
                # Image 0 is the kernel head: load in row-halves so the
                # binarize + first matmuls start before the full image lands.
                halves = ((0, 14), (14, 14), (28, 14), (42, 14)) if n == 0 else ((0, H),)
                xin = []
                xb = xbp.tile([128, 2, PLANE], dt.float8e4, tag="xb")
                for q in range(2):
                    xi = xinp.tile([128, HW], dt.float32, tag="xin")
                    xin.append(xi)
                    _zero_pads(nc, xb, q)
                for q in range(2):
                    for r0, nr in halves:
                        xi = xin[q]
                        nc.sync.dma_start(
                            xi[:, r0 * W:(r0 + nr) * W],
                            x_d[n, q * 128:(q + 1) * 128, r0:r0 + nr].rearrange(
                                "p h w -> p (h w)"),
                        )
                        nc.scalar.activation(
                            _plane_view(xb, q)[:, 1 + r0:1 + r0 + nr, 1:57],
                            xi[:, r0 * W:(r0 + nr) * W].rearrange(
                                "p (h w) -> p h w", w=W),
                            AF.Sign,
                        )

                # ---- conv1 -> sign(bn1(.)) into padded intermediate ----
                ab = abp.tile([128, 2, PLANE], dt.float8e4, tag="ab")
                for co in range(2):
                    _zero_pads(nc, ab, co)
                for co in range(2):
                    for s in range(NCH):
                        ps = psp.tile([128, CHU], dt.float32, tag="ps")
                        for kk in range(9):
                            off = (RPC * s + kk // 3) * WP + kk % 3
                            rhs = xb[:, :, off:off + CH].rearrange(
                                "p k (r c) -> p k r c", c=WP)[:, :, :, 0:W]
                            nc.tensor.matmul(
                                ps[:],
                                w_sb[0][:, :, kk, co * 128:(co + 1) * 128],
                                rhs,
                                start=(kk == 0),
                                stop=(kk == 8),
                                perf_mode=PM.DoubleRow,
                            )
                        psv = ps[:].rearrange("p (r c) -> p r c", c=W)
                        dst = _plane_view(ab, co)[:, 1 + RPC * s:1 + RPC * s + RPC, 1:57]
                        nc.scalar.activation(
                            dst, psv, AF.Sign,
                            bias=t1_sb[:, co:co + 1], scale=s1_sb[:, co:co + 1],
                        )

                # ---- conv2 -> bn2 + residual + clip ----
                for co in range(2):
                    ost = ostp.tile([128, HW], dt.float32, tag="ost")
                    ostv = ost[:].rearrange("p (h w) -> p h w", w=W)
                    xinv = xin[co][:].rearrange("p (h w) -> p h w", w=W)
                    for s in range(NCH):
                        ps = psp.tile([128, CHU], dt.float32, tag="ps")
                        for kk in range(9):
                            off = (RPC * s + kk // 3) * WP + kk % 3
                            rhs = ab[:, :, off:off + CH].rearrange(
                                "p k (r c) -> p k r c", c=WP)[:, :, :, 0:W]
                            nc.tensor.matmul(
                                ps[:],
                                w_sb[1][:, :, kk, co * 128:(co + 1) * 128],
                                rhs,
                                start=(kk == 0),
                                stop=(kk == 8),
                                perf_mode=PM.DoubleRow,
                            )
                        psv = ps[:].rearrange("p (r c) -> p r c", c=W)
                        tm = tmpp.tile([128, RPC * W], dt.float32, tag="tmp")
                        tmv = tm[:].rearrange("p (r c) -> p r c", c=W)
                        nc.scalar.activation(
                            tmv, psv, AF.Identity,
                            bias=t2_sb[:, co:co + 1], scale=s2_sb[:, co:co + 1],
                        )
                        ov = ostv[:, RPC * s:RPC * s + RPC, :]
                        nc.vector.tensor_tensor(
                            ov, tmv, xinv[:, RPC * s:RPC * s + RPC, :], ALU.add
                        )
                        nc.vector.tensor_scalar(ov, ov, 1.0, -1.0, ALU.min, ALU.max)
                        if s == 3:
                            nc.sync.dma_start(
                                out_d[n, co * 128:(co + 1) * 128, 0:32].rearrange(
                                    "p h w -> p (h w)"),
                                ost[:, 0:32 * W],
                            )
                        elif s >= 4:
                            r0, r1 = 8 * s, 8 * s + 8
                            nc.sync.dma_start(
                                out_d[n, co * 128:(co + 1) * 128, r0:r1].rearrange(
                                    "p h w -> p (h w)"),
                                ost[:, r0 * W:r1 * W],
                            )

    nc.compile()
    return nc


def _get_nc():
    if "nc" not in _CACHE:
        _CACHE["nc"] = _build()
    return _CACHE["nc"]


def _prep_weights(w):
    # [co, cin, kh, kw] -> [cin_chunk 2, cin 128, tap 9, co 256], binarized fp8e4
    a = np.sign(w.astype(np.float32))
    a = a.transpose(1, 2, 3, 0).reshape(2, 128, 9, C)
    return np.ascontiguousarray(a.astype(ml_dtypes.float8_e4m3))


def _fold_bn(g, b, m, v):
    s = (g.astype(np.float32) / np.sqrt(v.astype(np.float32) + BN_EPS)).astype(np.float32)
    t = (b.astype(np.float32) - m.astype(np.float32) * s).astype(np.float32)
    return (
        np.ascontiguousarray(s.reshape(2, 128)),
        np.ascontiguousarray(t.reshape(2, 128)),
    )


def kernel(x, w1, g1, b1, m1, v1, w2, g2, b2, m2, v2):
    nc = _get_nc()
    w1b = _prep_weights(w1)
    w2b = _prep_weights(w2)
    s1, t1 = _fold_bn(g1, b1, m1, v1)
    s2, t2 = _fold_bn(g2, b2, m2, v2)
    x = np.ascontiguousarray(x.astype(np.float32, copy=False))

    in_maps = []
    for c in range(N_CORES):
        in_maps.append({
            "x": x[c * NPER:(c + 1) * NPER],
            "w1b": w1b, "w2b": w2b,
            "s1": s1, "t1": t1, "s2": s2, "t2": t2,
        })
    res = run_bass_kernel_spmd(nc, in_maps, list(range(N_CORES)))
    out = np.concatenate([res.results[c]["out"] for c in range(N_CORES)], axis=0)
    return out


# revision 10
# speedup vs baseline: 1.0032x; 1.0032x over previous
"""Binarized BasicBlock (BNN) forward on 8 Trainium2 NeuronCores.

Reference computation (per reference.py):
    xb  = sign(x);  wb = sign(w)
    y1  = conv3x3(xb, wb1, pad=1)
    a1  = hardtanh(bn1(y1))          # only sign(a1) feeds conv2
    y2  = conv3x3(sign(a1), wb2, pad=1)
    out = hardtanh(bn2(y2) + x)

Strategy:
  - Data parallel: batch N=64 -> 8 images per core; weights/BN replicated.
  - Conv as 9 shifted matmuls over a zero-padded 58x58 image held in SBUF,
    contraction over input channels: 256 channels = 2 planes of 128
    partitions contracted in ONE matmul via fp8 DoubleRow perf mode.
  - Binarized operands stored as fp8e4 (+-1, 0 exact); PSUM accumulates
    fp32; sums of +-1 with <=2304 terms are exact integers in fp32.
  - BN folded into the activation op: sign(bn1(y)) = Sign(y*s1 + t1) with
    s1 = g1/sqrt(v1+eps), t1 = b1 - m1*s1 (host-folded, passed as inputs).
  - Final stage: Identity(y2*s2+t2) on ScalarE, then add-residual and
    clip (tensor_scalar min+max) on VectorE.
"""

import sys

try:
    import concourse  # noqa: F401
except ImportError:  # pragma: no cover
    sys.path.insert(0, "/opt/trn_rl_repo")

import numpy as np
import ml_dtypes

import concourse.bacc as bacc
import concourse.tile as tile
import concourse.mybir as mybir
from concourse.bass_utils import run_bass_kernel_spmd

dt = mybir.dt
AF = mybir.ActivationFunctionType
ALU = mybir.AluOpType
PM = mybir.MatmulPerfMode

N_CORES = 8
NPER = 8          # images per core
C = 256
H = W = 56
HW = H * W        # 3136
WP = 58           # padded row width (1 + 56 + 1)
ROWW = 64         # allocated width per (row, k-plane) block (16B aligned)
RPITCH = 2 * ROWW  # 128 = row pitch (both k-planes interleaved per row)
PROWS = 58        # padded rows
PLSZ = PROWS * RPITCH  # 7424 = padded image tile length
RPC = 8           # output rows per matmul chunk
CHU = RPC * W     # 448 = useful matmul free dim (garbage cols skipped)
NCH = H // RPC    # 7 chunks per image
BN_EPS = 1e-5

_CACHE = {}


def _zero_pads(nc, t):
    """Zero the padding cells of a [128, PLSZ] row-interleaved image tile.

    Layout: element (row r, k-plane k, col c) at r*RPITCH + k*ROWW + c;
    c=1..56 hold image cols 0..55, c=0 and c=57..63 are zero pads, rows
    0 and 57 are zero pad rows."""
    v = t[:]
    nc.gpsimd.memset(v[:, 0:RPITCH], 0.0)                      # top pad row
    nc.gpsimd.memset(v[:, 57 * RPITCH:PLSZ], 0.0)              # bottom pad row
    # per-block right pads c=57..63 plus the following block's c=0
    cols = v[:, 57:57 + 57 * RPITCH].rearrange("p (r k c) -> p r k c", k=2, c=ROWW)
    nc.gpsimd.memset(cols[:, :, :, 0:8], 0.0)


def _rview(t):
    # [128, PROWS, 2, ROWW]
    return t[:].rearrange("p (r k c) -> p r k c", k=2, c=ROWW)


def _build():
    nc = bacc.Bacc("TRN2", target_bir_lowering=False, debug=False)

    x_d = nc.dram_tensor("x", [NPER, C, H, W], dt.float32, kind="ExternalInput").ap()
    w1_d = nc.dram_tensor("w1b", [2, 128, 9, C], dt.float8e4, kind="ExternalInput").ap()
    w2_d = nc.dram_tensor("w2b", [2, 128, 9, C], dt.float8e4, kind="ExternalInput").ap()
    s1_d = nc.dram_tensor("s1", [2, 128], dt.float32, kind="ExternalInput").ap()
    t1_d = nc.dram_tensor("t1", [2, 128], dt.float32, kind="ExternalInput").ap()
    s2_d = nc.dram_tensor("s2", [2, 128], dt.float32, kind="ExternalInput").ap()
    t2_d = nc.dram_tensor("t2", [2, 128], dt.float32, kind="ExternalInput").ap()
    out_d = nc.dram_tensor("out", [NPER, C, H, W], dt.float32, kind="ExternalOutput").ap()

    with tile.TileContext(nc) as tc:
        with (
            tc.tile_pool(name="wp", bufs=1) as wp,
            tc.tile_pool(name="xin", bufs=4) as xinp,
            tc.tile_pool(name="xb", bufs=2) as xbp,
            tc.tile_pool(name="ab", bufs=2) as abp,
            tc.tile_pool(name="ost", bufs=3) as ostp,
            tc.tile_pool(name="tmp", bufs=4) as tmpp,
            tc.tile_pool(name="ps", bufs=6, space="PSUM") as psp,
            nc.sbuf_tensor([128, 2 * CHU], dt.float8e4) as warm_in,
            nc.psum_tensor([128, CHU], dt.float32) as warm_ps,
        ):
            # ---- PE warm-up: junk matmuls on scratch data so HAM reaches
            # 8/8 before the first real matmul and the PE never starts cold.
            nc.gpsimd.memset(warm_in[:], 0.0)
            wv = warm_in[:].rearrange("p (k c) -> p k c", k=2)
            for _ in range(18):
                nc.tensor.matmul(
                    warm_ps[:], wv[:, :, 0:128], wv[:],
                    start=True, stop=True, perf_mode=PM.DoubleRow,
                )

            # Weight/BN loads go on the scalar HWDGE queue so the sync queue
            # starts with image 0's input (the critical path).
            w_sb = []
            for wd, tag in ((w1_d, "w1"), (w2_d, "w2")):
                t = wp.tile([128, 2, 9, C], dt.float8e4, tag=tag)
                nc.scalar.dma_start(t[:], wd.rearrange("q p k c -> p q k c"))
                w_sb.append(t)
            bn_sb = []
            for bd, tag in ((s1_d, "s1"), (t1_d, "t1"), (s2_d, "s2"), (t2_d, "t2")):
                t = wp.tile([128, 2], dt.float32, tag=tag)
                nc.scalar.dma_start(t[:], bd.rearrange("q p -> p q"))
                bn_sb.append(t)
            s1_sb, t1_sb, s2_sb, t2_sb = bn_sb

            for n in range(NPER):
                # ---- load + binarize input ----
                # Image 0 is the kernel head: quarter-granular loads on two
                # DMA queues; the row-interleaved layout keeps matmul AP
                # extents tight so the first chunks start after ~10 rows.
                quarts = ((0, 14), (14, 14), (28, 14), (42, 14)) if n == 0 else ((0, H),)
                xin = []
                xb = xbp.tile([128, PLSZ], dt.float8e4, tag="xb")
                _zero_pads(nc, xb)
                xbv = _rview(xb)
                for q in range(2):
                    xi = xinp.tile([128, HW], dt.float32, tag="xin")
                    xin.append(xi)
                for r0, nr in quarts:
                    for q in range(2):
                        xi = xin[q]
                        dma_eng = nc.gpsimd if (n == 0 and q == 1) else nc.sync
                        dma_eng.dma_start(
                            xi[:, r0 * W:(r0 + nr) * W],
                            x_d[n, q * 128:(q + 1) * 128, r0:r0 + nr].rearrange(
                                "p h w -> p (h w)"),
                        )
                        nc.scalar.activation(
                            xbv[:, 1 + r0:1 + r0 + nr, q, 1:57],
                            xi[:, r0 * W:(r0 + nr) * W].rearrange(
                                "p (h w) -> p h w", w=W),
                            AF.Sign,
                        )

                # ---- conv1 -> sign(bn1(.)) into padded intermediate ----
                ab = abp.tile([128, PLSZ], dt.float8e4, tag="ab")
                _zero_pads(nc, ab)
                abv = _rview(ab)
                for co in range(2):
                    for s in range(NCH):
                        ps = psp.tile([128, CHU], dt.float32, tag="ps")
                        for kk in range(9):
                            r0 = RPC * s + kk // 3
                            rhs = xbv[:, r0:r0 + RPC, :, kk % 3:kk % 3 + W].rearrange(
                                "p r k c -> p k r c")
                            nc.tensor.matmul(
                                ps[:],
                                w_sb[0][:, :, kk, co * 128:(co + 1) * 128],
                                rhs,
                                start=(kk == 0),
                                stop=(kk == 8),
                                perf_mode=PM.DoubleRow,
                            )
                        psv = ps[:].rearrange("p (r c) -> p r c", c=W)
                        nc.scalar.activation(
                            abv[:, 1 + RPC * s:1 + RPC * s + RPC, co, 1:57], psv, AF.Sign,
                            bias=t1_sb[:, co:co + 1], scale=s1_sb[:, co:co + 1],
                        )

                # ---- conv2 -> bn2 + residual + clip ----
                for co in range(2):
                    ost = ostp.tile([128, HW], dt.float32, tag="ost")
                    ostv = ost[:].rearrange("p (h w) -> p h w", w=W)
                    xinv = xin[co][:].rearrange("p (h w) -> p h w", w=W)
                    for s in range(NCH):
                        ps = psp.tile([128, CHU], dt.float32, tag="ps")
                        for kk in range(9):
                            r0 = RPC * s + kk // 3
                            rhs = abv[:, r0:r0 + RPC, :, kk % 3:kk % 3 + W].rearrange(
                                "p r k c -> p k r c")
                            nc.tensor.matmul(
                                ps[:],
                                w_sb[1][:, :, kk, co * 128:(co + 1) * 128],
                                rhs,
                                start=(kk == 0),
                                stop=(kk == 8),
                                perf_mode=PM.DoubleRow,
                            )
                        psv = ps[:].rearrange("p (r c) -> p r c", c=W)
                        tm = tmpp.tile([128, RPC * W], dt.float32, tag="tmp")
                        tmv = tm[:].rearrange("p (r c) -> p r c", c=W)
                        nc.scalar.activation(
                            tmv, psv, AF.Identity,
                            bias=t2_sb[:, co:co + 1], scale=s2_sb[:, co:co + 1],
                        )
                        ov = ostv[:, RPC * s:RPC * s + RPC, :]
                        nc.vector.tensor_tensor(
                            ov, tmv, xinv[:, RPC * s:RPC * s + RPC, :], ALU.add
                        )
                        nc.vector.tensor_scalar(ov, ov, 1.0, -1.0, ALU.min, ALU.max)
                        if s == 3:
                            nc.sync.dma_start(
                                out_d[n, co * 128:(co + 1) * 128, 0:32].rearrange(
                                    "p h w -> p (h w)"),
                                ost[:, 0:32 * W],
                            )
                        elif s >= 4:
                            r0o, r1o = 8 * s, 8 * s + 8
                            nc.sync.dma_start(
                                out_d[n, co * 128:(co + 1) * 128, r0o:r1o].rearrange(
                                    "p h w -> p (h w)"),
                                ost[:, r0o * W:r1o * W],
                            )

    nc.compile()
    return nc


def _get_nc():
    if "nc" not in _CACHE:
        _CACHE["nc"] = _build()
    return _CACHE["nc"]


def _prep_weights(w):
    # [co, cin, kh, kw] -> [cin_chunk 2, cin 128, tap 9, co 256], binarized fp8e4
    a = np.sign(w.astype(np.float32))
    a = a.transpose(1, 2, 3, 0).reshape(2, 128, 9, C)
    return np.ascontiguousarray(a.astype(ml_dtypes.float8_e4m3))


def _fold_bn(g, b, m, v):
    s = (g.astype(np.float32) / np.sqrt(v.astype(np.float32) + BN_EPS)).astype(np.float32)
    t = (b.astype(np.float32) - m.astype(np.float32) * s).astype(np.float32)
    return (
        np.ascontiguousarray(s.reshape(2, 128)),
        np.ascontiguousarray(t.reshape(2, 128)),
    )


def kernel(x, w1, g1, b1, m1, v1, w2, g2, b2, m2, v2):
    nc = _get_nc()
    w1b = _prep_weights(w1)
    w2b = _prep_weights(w2)
    s1, t1 = _fold_bn(g1, b1, m1, v1)
    s2, t2 = _fold_bn(g2, b2, m2, v2)
    x = np.ascontiguousarray(x.astype(np.float32, copy=False))

    in_maps = []
    for c in range(N_CORES):
        in_maps.append({
            "x": x[c * NPER:(c + 1) * NPER],
            "w1b": w1b, "w2b": w2b,
            "s1": s1, "t1": t1, "s2": s2, "t2": t2,
        })
    res = run_bass_kernel_spmd(nc, in_maps, list(range(N_CORES)))
    out = np.concatenate([res.results[c]["out"] for c in range(N_CORES)], axis=0)
    return out
